# revision 12
# baseline (speedup 1.0000x reference)
# Multi-head attention (b=2, n=2048, d_model=1024, 16 heads) on 8 NeuronCores.
#
# Sharding: core c = (batch b, head-group g) with b = c//4, g = c%4.
# Each core handles 1 batch element and 4 heads (256 channels), computing a
# partial output projection; the host sums the 4 group-partials per batch and
# adds b_O.
#
# v2 design (scalar-exp-bound schedule, ~147us EXP floor):
#  - Heads processed in PAIRS (cs in {0,1}; rows 0:64 / 64:128 of qt/kt[cs]).
#    The two score matmuls of a pair have K=64 and auto-derive PE row-tiles
#    (0,0)/(64,0) from their base partitions -> they stream CONCURRENTLY.
#  - Query chunks of 512; st pair-packed [128, 1024] (h_even | h_odd) in PSUM,
#    double-buffered; ONE [128,1024] Exp per (pair, m-slice) on ScalarE with
#    the 1/8 scale folded in, output DIRECTLY in fp8e4.
#  - A*V runs in fp8 DoubleRow: Ko=2 packs consecutive m-slices, so each
#    matmul streams 2 slices worth of E (half the PE stream time of bf16).
#    V is stored fp8 as v4p[mp] = [128, (ko=2, h=4, 72)] with a ones column
#    at offset 64 (softmax denominators fall out of PSUM row 64 for free).
#    (fp8 on E/V measured 1.7e-2 rel err vs the 2e-2 gate in host sim;
#    projections/scores stay bf16 - fp8 there blows the budget.)
#  - Segments run PAIR-MAJOR (all 4 chunks of pair 0, then pair 1) so kt[1]
#    isn't needed until slot 64. Q/K/V/O projection chains are deadline-paced
#    fillers eating PE idle under the scalar-bound attention loop; the et ring
#    (8 groups) lets A*V lag fillers without stalling ScalarE.

import ml_dtypes
import numpy as np

import concourse.bass as bass
import concourse.bacc as bacc
import concourse.tile as tile
from concourse import mybir
from concourse.bass_utils import run_bass_kernel_spmd

D = 1024  # d_model
N = 2048  # sequence length
B = 2  # batch
NHEADS = 16
DK = 64
NCORES = 8
GROUPS = 4  # head-groups across cores
HPG = NHEADS // GROUPS  # 4 heads per group
CH = HPG * DK  # 256 channels per group
KT = D // 128  # 8 contraction tiles for the projections
MS = N // 128  # 16 m-slices (key dim)
MP = MS // 2  # 8 m-slice pairs (DoubleRow Ko=2)
NCHUNK = 512  # query-chunk width
NCHUNKS = N // NCHUNK
VPITCH = 72  # per-head pitch in v4p (65 used, pad so ko-stride % 16 == 0)

F32 = mybir.dt.float32
BF16 = mybir.dt.bfloat16
FP8 = mybir.dt.float8e4


def _build_bass():
    nc = bacc.Bacc()

    xT_d = nc.dram_tensor("xT", [D, N], BF16, kind="ExternalInput")
    wqT_d = nc.dram_tensor("wqT", [D, CH], BF16, kind="ExternalInput")
    wkT_d = nc.dram_tensor("wkT", [D, CH], BF16, kind="ExternalInput")
    wvT_d = nc.dram_tensor("wvT", [D, CH], BF16, kind="ExternalInput")
    woT_d = nc.dram_tensor("woT", [CH, D], BF16, kind="ExternalInput")
    bq_d = nc.dram_tensor("bq", [CH], F32, kind="ExternalInput")
    bk_d = nc.dram_tensor("bk", [CH], F32, kind="ExternalInput")
    bv_d = nc.dram_tensor("bv", [CH], F32, kind="ExternalInput")
    vones_d = nc.dram_tensor("vones", [128, 2 * HPG], FP8, kind="ExternalInput")
    yT_d = nc.dram_tensor("yT", [D, N], F32, kind="ExternalOutput")

    with tile.TileContext(nc) as tc:
        with (
            tc.tile_pool(name="persist", bufs=1) as persist,
            tc.tile_pool(name="et_pool", bufs=8) as et_pool,
            tc.tile_pool(name="osb_pool", bufs=1) as osb_pool,
            tc.tile_pool(name="small", bufs=2) as small,
            tc.tile_pool(name="aux_ps", bufs=2, space="PSUM") as aux_ps,
            tc.tile_pool(name="st_ps", bufs=2, space="PSUM") as st_pool,
            tc.tile_pool(name="ot_ps", bufs=1, space="PSUM") as ot_pool,
        ):
            # ---- input loads. Per-core IO is DMA-latency critical at the
            # front: the first K/Q chains only need xT's first 512 COLUMNS,
            # so stream xT in column blocks (weights first, ~0.5MB, then
            # 1MB per column block) instead of whole k-tiles.
            xt, wq, wk, wv = [], [], [], []
            for k in range(KT):
                t = persist.tile([128, N], BF16, tag=f"xt{k}", name=f"xt{k}")
                xt.append(t)
                for wname, dram, lst in (("wk", wkT_d, wk), ("wq", wqT_d, wq)):
                    t = persist.tile([128, CH], BF16, tag=f"{wname}{k}", name=f"{wname}{k}")
                    nc.sync.dma_start(out=t, in_=dram[k * 128 : (k + 1) * 128, :])
                    lst.append(t)
            bq_t, bk_t = [], []
            for bname, dram, lst in (("bq", bq_d, bq_t), ("bk", bk_d, bk_t)):
                for cs in range(CH // 128):
                    t = persist.tile([128, 1], F32, tag=f"{bname}{cs}", name=f"{bname}{cs}")
                    nc.sync.dma_start(out=t, in_=dram[cs * 128 : (cs + 1) * 128])
                    lst.append(t)
            for k in range(KT):  # xT column block 0
                nc.sync.dma_start(out=xt[k][:, 0:512], in_=xT_d[k * 128 : (k + 1) * 128, 0:512])
            for k in range(KT):
                t = persist.tile([128, CH], BF16, tag=f"wv{k}", name=f"wv{k}")
                nc.sync.dma_start(out=t, in_=wvT_d[k * 128 : (k + 1) * 128, :])
                wv.append(t)
            for nb in range(1, 4):  # remaining xT column blocks
                for k in range(KT):
                    nc.sync.dma_start(
                        out=xt[k][:, nb * 512 : (nb + 1) * 512],
                        in_=xT_d[k * 128 : (k + 1) * 128, nb * 512 : (nb + 1) * 512],
                    )
            bvb = persist.tile([128, CH], F32, tag="bvb", name="bvb")
            bv_ap = bv_d[None, :]
            nc.gpsimd.dma_start(
                out=bvb,
                in_=bass.AP(tensor=bv_ap.tensor, offset=bv_ap.offset, ap=[[0, 128]] + list(bv_ap.ap[1:])),
            )

            # ---- persistent tensors ----
            qt = [persist.tile([128, N], BF16, tag=f"qt{cs}", name=f"qt{cs}") for cs in range(CH // 128)]
            kt = [persist.tile([128, N], BF16, tag=f"kt{cs}", name=f"kt{cs}") for cs in range(CH // 128)]
            # v4p[mp]: fp8, layout [128, (ko=2, h=4, VPITCH)]; per head cols
            # h*VPITCH .. +64 = V channels, col 64 = ones (denominator trick)
            v4p = [persist.tile([128, 2 * HPG * VPITCH], FP8, tag=f"v4p{mp}", name=f"v4p{mp}") for mp in range(MP)]
            wot = []
            for cs in range(CH // 128):
                t = persist.tile([128, D], BF16, tag=f"wot{cs}", name=f"wot{cs}")
                nc.sync.dma_start(out=t, in_=woT_d[cs * 128 : (cs + 1) * 128, :])
                wot.append(t)
            osb = {}
            for c in range(NCHUNKS):
                for cs in range(CH // 128):
                    osb[(c, cs)] = osb_pool.tile(
                        [128, NCHUNK], BF16, tag=f"osb{c}_{cs}", name=f"osb{c}_{cs}"
                    )

            # ---- filler emitters (projection chains on aux PSUM), split in
            # half-chain units so a filler never delays the next score
            # matmuls (and hence ScalarE) by more than ~0.6us of PE time.
            pend = {}

            def emit_v(ms, half):
                mp, ko = divmod(ms, 2)
                if half == 0:
                    ps = aux_ps.tile([128, 512], F32, tag="aux", name="aux_ps_t", bufs=2)
                    pend[("v", ms)] = ps
                else:
                    ps = pend.pop(("v", ms))
                ks = range(0, KT // 2) if half == 0 else range(KT // 2, KT)
                for k in ks:
                    nc.tensor.matmul(
                        ps[:, 0:CH],
                        xt[k][:, ms * 128 : (ms + 1) * 128],
                        wv[k],
                        start=(k == 0),
                        stop=(k == KT - 1),
                    )
                if half == 0:
                    return
                v4v = v4p[mp].rearrange("p (k h s) -> p k h s", k=2, h=HPG)
                if ko == 0:
                    nc.sync.dma_start(out=v4v[:, :, :, 64:65], in_=vones_d[:, :])
                nc.vector.tensor_add(
                    out=v4v[:, ko, :, 0:64],
                    in0=ps[:, 0:CH].rearrange("p (h c) -> p h c", c=64),
                    in1=bvb.rearrange("p (h c) -> p h c", c=64),
                )

            def emit_qk_chain(isq, cs, n0, half=None):
                dst, w, bias = (qt, wq, bq_t) if isq else (kt, wk, bk_t)
                if half == 1:
                    ps = pend.pop(("qk", isq, cs, n0))
                else:
                    ps = aux_ps.tile([128, 512], F32, tag="aux", name="aux_ps_t", bufs=2)
                    if half == 0:
                        pend[("qk", isq, cs, n0)] = ps
                halves = (0, 1) if half is None else (half,)
                for h2 in halves:
                    for k in range(h2 * KT // 2, (h2 + 1) * KT // 2):
                        nc.tensor.matmul(
                            ps,
                            w[k][:, cs * 128 : (cs + 1) * 128],
                            xt[k][:, n0 : n0 + 512],
                            start=(k == 0),
                            stop=(k == KT - 1),
                        )
                if half == 0:
                    return
                nc.vector.tensor_scalar_add(
                    out=dst[cs][:, n0 : n0 + 512], in0=ps, scalar1=bias[cs]
                )

            def emit_f(c, msl, half=None):
                if half == 1:
                    yp = pend.pop(("f", c, msl))
                else:
                    yp = aux_ps.tile([128, 512], F32, tag="aux", name="aux_yt_t", bufs=2)  # shares the "aux" ring; segs 5-7 run only emit_f
                    if half == 0:
                        pend[("f", c, msl)] = yp
                halves = (0, 1) if half is None else (half,)
                for cs in halves:
                    nc.tensor.matmul(
                        yp,
                        wot[cs][:, msl * 128 : (msl + 1) * 128],
                        osb[(c, cs)],
                        start=(cs == 0),
                        stop=(cs == CH // 128 - 1),
                    )
                if half == 0:
                    return
                ysb = small.tile([128, 512], F32, tag="ysb", name="ysb_t", bufs=4)
                nc.vector.tensor_copy(out=ysb, in_=yp)
                nc.sync.dma_start(
                    out=yT_d[msl * 128 : (msl + 1) * 128, c * NCHUNK : (c + 1) * NCHUNK],
                    in_=ysb,
                )

            # ---- prelude: just what segment 0 needs to start ----
            emit_qk_chain(False, 0, 0)  # kt[0] cols 0:512 (ms 0..3)
            emit_qk_chain(True, 0, 0)   # qt[0] cols 0:512 (chunk 0)

            # ---- deadline-paced fillers, emitted AFTER scores+exp of their
            # slot (so score matmuls always lead in PE queue order) and
            # BEFORE the A*V matmul of their m-slice pair (so v4p[mp] writes
            # precede the DoubleRow matmul that reads them - Tile derives
            # dependencies from program order).
            def V(ms, h2):
                return lambda: emit_v(ms, h2)

            def QK(isq, cs, n0, h2):
                return lambda: emit_qk_chain(isq, cs, n0, h2)

            def F(c, msl, h2):
                return lambda: emit_f(c, msl, h2)

            seg_fill = {
                0: {
                    0: [V(0, 0), V(0, 1)],
                    1: [V(1, 0), V(1, 1)],
                    2: [V(2, 0), V(2, 1), QK(False, 0, 512, 0)],
                    3: [V(3, 0), V(3, 1), QK(False, 0, 512, 1)],
                    4: [V(4, 0), V(4, 1)],
                    5: [V(5, 0), V(5, 1)],
                    6: [V(6, 0), V(6, 1), QK(False, 0, 1024, 0)],
                    7: [V(7, 0), V(7, 1), QK(False, 0, 1024, 1)],
                    8: [V(8, 0), V(8, 1)],
                    9: [V(9, 0), V(9, 1)],
                    10: [V(10, 0), V(10, 1), QK(False, 0, 1536, 0)],
                    11: [V(11, 0), V(11, 1), QK(False, 0, 1536, 1)],
                    12: [V(12, 0), V(12, 1)],
                    13: [V(13, 0), V(13, 1)],
                    14: [V(14, 0), V(14, 1), QK(True, 0, 512, 0)],
                    15: [V(15, 0), V(15, 1), QK(True, 0, 512, 1)],
                },
                1: {
                    1: [QK(True, 0, 1024, 0)],
                    2: [QK(True, 0, 1024, 1)],
                    3: [QK(False, 1, 0, 0)],
                    4: [QK(False, 1, 0, 1)],
                    5: [QK(False, 1, 512, 0)],
                    6: [QK(False, 1, 512, 1)],
                    7: [QK(True, 0, 1536, 0)],
                    8: [QK(True, 0, 1536, 1)],
                    9: [QK(False, 1, 1024, 0)],
                    10: [QK(False, 1, 1024, 1)],
                    11: [QK(False, 1, 1536, 0)],
                    12: [QK(False, 1, 1536, 1)],
                },
                2: {
                    1: [QK(True, 1, 0, 0)],
                    2: [QK(True, 1, 0, 1)],
                    5: [QK(True, 1, 512, 0)],
                    6: [QK(True, 1, 512, 1)],
                },
                3: {
                    1: [QK(True, 1, 1024, 0)],
                    2: [QK(True, 1, 1024, 1)],
                    5: [QK(True, 1, 1536, 0)],
                    6: [QK(True, 1, 1536, 1)],
                },
                4: {},
                5: {s: [F(0, s // 2, s % 2)] for s in range(16)},
                6: {s: [F(1, s // 2, s % 2)] for s in range(16)},
                7: {s: [F(2, s // 2, s % 2)] for s in range(16)},
            }

            # ---- attention: pair-major segments ----
            seg = 0
            for cs in range(2):
                for c in range(NCHUNKS):
                    n0 = c * NCHUNK
                    fillers = seg_fill[seg]
                    ot = [
                        ot_pool.tile([65, NCHUNK], F32, tag=f"ot{hi}", name=f"ot{hi}_t")
                        for hi in range(2)
                    ]
                    for mp in range(MP):
                        et = et_pool.tile([128, 2048], FP8, tag="et", name="et_t")
                        for mi in range(2):
                            ms = 2 * mp + mi
                            st = st_pool.tile([128, 1024], F32, tag="st", name="st_t")
                            for hi in range(2):
                                r0 = hi * 64
                                nc.tensor.matmul(
                                    st[:, hi * 512 : (hi + 1) * 512],
                                    kt[cs][r0 : r0 + 64, ms * 128 : (ms + 1) * 128],
                                    qt[cs][r0 : r0 + 64, n0 : n0 + 512],
                                    start=True,
                                    stop=True,
                                )
                            nc.scalar.activation(
                                out=et[:, mi * 1024 : (mi + 1) * 1024],
                                in_=st,
                                func=mybir.ActivationFunctionType.Exp,
                                scale=float(1.0 / np.sqrt(DK)),
                            )
                            for f in fillers.get(2 * mp + mi, []):
                                f()
                        etv = et.rearrange("p (k n) -> p k n", k=2)
                        v4v = v4p[mp].rearrange("p (k s) -> p k s", k=2)
                        for hi in range(2):
                            h = 2 * cs + hi
                            nc.tensor.matmul(
                                ot[hi],
                                v4v[:, :, h * VPITCH : h * VPITCH + 65],
                                etv[:, :, hi * 512 : (hi + 1) * 512],
                                start=(mp == 0),
                                stop=(mp == MP - 1),
                                perf_mode=mybir.MatmulPerfMode.DoubleRow,
                            )
                    # normalize: drain ot, reciprocal of row-64 sums via a
                    # [128, 4] reshuffle, broadcast, multiply into osb rows.
                    for hi in range(2):
                        oraw = small.tile([65, NCHUNK], F32, tag="oraw", name="oraw_t")
                        nc.vector.tensor_copy(out=oraw, in_=ot[hi])
                        rcin = small.tile([128, NCHUNK // 128], F32, tag="rcin", name="rcin_t")
                        nc.sync.dma_start(out=rcin, in_=oraw[64:65, :])
                        rc = small.tile([128, NCHUNK // 128], F32, tag="rc", name="rc_t")
                        nc.vector.reciprocal(out=rc, in_=rcin)
                        rflat = small.tile([1, NCHUNK], F32, tag="rflat", name="rflat_t")
                        nc.sync.dma_start(out=rflat, in_=rc)
                        rb = small.tile([128, NCHUNK], F32, tag="rb", name="rb_t")
                        nc.gpsimd.partition_broadcast(rb, rflat)
                        nc.vector.tensor_mul(
                            out=osb[(c, cs)][hi * 64 : (hi + 1) * 64, :],
                            in0=oraw[0:64, :],
                            in1=rb[0:64, :],
                        )
                    seg += 1
            # epilogue: last chunk's output projection
            for msl in range(D // 128):
                emit_f(3, msl)
    nc.compile()
    return nc


_NC = None


def _get_nc():
    global _NC
    if _NC is None:
        _NC = _build_bass()
    return _NC


def build_in_maps(inputs):
    x = np.asarray(inputs["x"], dtype=np.float32)
    W_Q = np.asarray(inputs["W_Q"], dtype=np.float32)
    W_K = np.asarray(inputs["W_K"], dtype=np.float32)
    W_V = np.asarray(inputs["W_V"], dtype=np.float32)
    W_O = np.asarray(inputs["W_O"], dtype=np.float32)
    b_Q = np.asarray(inputs["b_Q"], dtype=np.float32)
    b_K = np.asarray(inputs["b_K"], dtype=np.float32)
    b_V = np.asarray(inputs["b_V"], dtype=np.float32)

    in_maps = []
    for core in range(NCORES):
        b, g = divmod(core, GROUPS)
        sl = slice(g * CH, (g + 1) * CH)
        in_maps.append(
            {
                "xT": np.ascontiguousarray(x[b].T.astype(ml_dtypes.bfloat16)),
                "wqT": np.ascontiguousarray(W_Q[sl, :].T.astype(ml_dtypes.bfloat16)),
                "wkT": np.ascontiguousarray(W_K[sl, :].T.astype(ml_dtypes.bfloat16)),
                "wvT": np.ascontiguousarray(W_V[sl, :].T.astype(ml_dtypes.bfloat16)),
                "woT": np.ascontiguousarray(W_O[:, sl].T.astype(ml_dtypes.bfloat16)),
                "bq": np.ascontiguousarray(b_Q[sl]),
                "bk": np.ascontiguousarray(b_K[sl]),
                "bv": np.ascontiguousarray(b_V[sl]),
                "vones": np.ones((128, 2 * HPG), dtype=ml_dtypes.float8_e4m3),
            }
        )
    return in_maps


def kernel(**inputs):
    in_maps = build_in_maps(inputs)
    nc = _get_nc()
    res = run_bass_kernel_spmd(nc, in_maps, core_ids=list(range(NCORES)))

    b_O = np.asarray(inputs["b_O"], dtype=np.float32)
    out = np.zeros((B, N, D), dtype=np.float32)
    for core in range(NCORES):
        b = core // GROUPS
        out[b] += res.results[core]["yT"].T
    out += b_O
    return out


# revision 18
# speedup vs baseline: 1.1792x; 1.1792x over previous
# Multi-head attention (b=2, n=2048, d_model=1024, 16 heads) on 8 NeuronCores.
#
# Sharding: core c = (batch b, head-group g) with b = c//4, g = c%4.
# Each core handles 1 batch element and 4 heads (256 channels), computing a
# partial output projection; the host sums the 4 group-partials per batch and
# adds b_O.
#
# v2 design (scalar-exp-bound schedule, ~147us EXP floor):
#  - Heads processed in PAIRS (cs in {0,1}; rows 0:64 / 64:128 of qt/kt[cs]).
#    The two score matmuls of a pair have K=64 and auto-derive PE row-tiles
#    (0,0)/(64,0) from their base partitions -> they stream CONCURRENTLY.
#  - Query chunks of 512; st pair-packed [128, 1024] (h_even | h_odd) in PSUM,
#    double-buffered; ONE [128,1024] Exp per (pair, m-slice) on ScalarE with
#    the 1/8 scale folded in, output DIRECTLY in fp8e4.
#  - A*V runs in fp8 DoubleRow: Ko=2 packs consecutive m-slices, so each
#    matmul streams 2 slices worth of E (half the PE stream time of bf16).
#    V is stored fp8 as v4p[mp] = [128, (ko=2, h=4, 72)] with a ones column
#    at offset 64 (softmax denominators fall out of PSUM row 64 for free).
#    (fp8 on E/V measured 1.7e-2 rel err vs the 2e-2 gate in host sim;
#    projections/scores stay bf16 - fp8 there blows the budget.)
#  - Segments run PAIR-MAJOR (all 4 chunks of pair 0, then pair 1) so kt[1]
#    isn't needed until slot 64. Q/K/V/O projection chains are deadline-paced
#    fillers eating PE idle under the scalar-bound attention loop; the et ring
#    (8 groups) lets A*V lag fillers without stalling ScalarE.

import ml_dtypes
import numpy as np

import concourse.bass as bass
import concourse.bacc as bacc
import concourse.tile as tile
from concourse import mybir
from concourse.bass_utils import run_bass_kernel_spmd

D = 1024  # d_model
N = 2048  # sequence length
B = 2  # batch
NHEADS = 16
DK = 64
NCORES = 8
GROUPS = 4  # head-groups across cores
HPG = NHEADS // GROUPS  # 4 heads per group
CH = HPG * DK  # 256 channels per group
KT = D // 128  # 8 contraction tiles for the projections
MS = N // 128  # 16 m-slices (key dim)
MP = MS // 2  # 8 m-slice pairs (DoubleRow Ko=2)
NCHUNK = 512  # query-chunk width
NCHUNKS = N // NCHUNK
VPITCH = 72  # per-head pitch in v4p (65 used, pad so ko-stride % 16 == 0)

F32 = mybir.dt.float32
F16 = mybir.dt.float16
BF16 = mybir.dt.bfloat16
FP8 = mybir.dt.float8e4


def _build_bass():
    nc = bacc.Bacc()

    xT_d = nc.dram_tensor("xT", [D, N], BF16, kind="ExternalInput")
    wqT_d = nc.dram_tensor("wqT", [D, CH], BF16, kind="ExternalInput")
    wkT_d = nc.dram_tensor("wkT", [D, CH], BF16, kind="ExternalInput")
    wvT_d = nc.dram_tensor("wvT", [D, CH], BF16, kind="ExternalInput")
    woT_d = nc.dram_tensor("woT", [CH, D], BF16, kind="ExternalInput")
    bq_d = nc.dram_tensor("bq", [CH], F32, kind="ExternalInput")
    bk_d = nc.dram_tensor("bk", [CH], F32, kind="ExternalInput")
    bv_d = nc.dram_tensor("bv", [CH], F32, kind="ExternalInput")
    vones_d = nc.dram_tensor("vones", [128, 2 * HPG], FP8, kind="ExternalInput")
    # f16 output (10 mantissa bits, ~4x tighter than bf16; values << f16
    # range): halves the 8MB/core output DMA, which is descriptor-rate bound.
    yT_d = nc.dram_tensor("yT", [D, N], F16, kind="ExternalOutput")

    with tile.TileContext(nc) as tc:
        with (
            tc.tile_pool(name="persist", bufs=1) as persist,
            tc.tile_pool(name="et_pool", bufs=8) as et_pool,
            tc.tile_pool(name="osb_pool", bufs=1) as osb_pool,
            tc.tile_pool(name="small", bufs=2) as small,
            tc.tile_pool(name="aux_ps", bufs=2, space="PSUM") as aux_ps,
            tc.tile_pool(name="st_ps", bufs=2, space="PSUM") as st_pool,
            tc.tile_pool(name="ot_ps", bufs=1, space="PSUM") as ot_pool,
        ):
            # ---- input loads. Per-core IO is DMA-latency critical at the
            # front: the first K/Q chains only need xT's first 512 COLUMNS,
            # so stream xT in column blocks (weights first, ~0.5MB, then
            # 1MB per column block) instead of whole k-tiles.
            xt, wq, wk, wv = [], [], [], []
            for k in range(KT):  # wk + xT block 0 interleaved: exactly what the first K chain consumes
                t = persist.tile([128, N], BF16, tag=f"xt{k}", name=f"xt{k}")
                xt.append(t)
                t = persist.tile([128, CH], BF16, tag=f"wk{k}", name=f"wk{k}")
                nc.sync.dma_start(out=t, in_=wkT_d[k * 128 : (k + 1) * 128, :])
                wk.append(t)
                nc.sync.dma_start(out=xt[k][:, 0:512], in_=xT_d[k * 128 : (k + 1) * 128, 0:512])
            bq_t, bk_t = [], []
            for bname, dram, lst in (("bq", bq_d, bq_t), ("bk", bk_d, bk_t)):
                for cs in range(CH // 128):
                    t = persist.tile([128, 1], F32, tag=f"{bname}{cs}", name=f"{bname}{cs}")
                    nc.sync.dma_start(out=t, in_=dram[cs * 128 : (cs + 1) * 128])
                    lst.append(t)
            for k in range(KT):
                t = persist.tile([128, CH], BF16, tag=f"wq{k}", name=f"wq{k}")
                nc.sync.dma_start(out=t, in_=wqT_d[k * 128 : (k + 1) * 128, :])
                wq.append(t)
            for k in range(KT):  # xT column block 1 (V chains for ms>=4 need it early in seg 0)
                nc.sync.dma_start(out=xt[k][:, 512:1024], in_=xT_d[k * 128 : (k + 1) * 128, 512:1024])
            for k in range(KT):
                t = persist.tile([128, CH], BF16, tag=f"wv{k}", name=f"wv{k}")
                nc.sync.dma_start(out=t, in_=wvT_d[k * 128 : (k + 1) * 128, :])
                wv.append(t)
            for nb in range(2, 4):
                for k in range(KT):
                    nc.sync.dma_start(
                        out=xt[k][:, nb * 512 : (nb + 1) * 512],
                        in_=xT_d[k * 128 : (k + 1) * 128, nb * 512 : (nb + 1) * 512],
                    )
            bvb = persist.tile([128, CH], F32, tag="bvb", name="bvb")
            bv_ap = bv_d[None, :]
            nc.gpsimd.dma_start(
                out=bvb,
                in_=bass.AP(tensor=bv_ap.tensor, offset=bv_ap.offset, ap=[[0, 128]] + list(bv_ap.ap[1:])),
            )

            # ---- persistent tensors ----
            qt = [persist.tile([128, N], BF16, tag=f"qt{cs}", name=f"qt{cs}") for cs in range(CH // 128)]
            kt = [persist.tile([128, N], BF16, tag=f"kt{cs}", name=f"kt{cs}") for cs in range(CH // 128)]
            # v4p[mp]: fp8, layout [128, (ko=2, h=4, VPITCH)]; per head cols
            # h*VPITCH .. +64 = V channels, col 64 = ones (denominator trick)
            v4p = [persist.tile([128, 2 * HPG * VPITCH], FP8, tag=f"v4p{mp}", name=f"v4p{mp}") for mp in range(MP)]
            wot = []
            for cs in range(CH // 128):
                t = persist.tile([128, D], BF16, tag=f"wot{cs}", name=f"wot{cs}")
                nc.sync.dma_start(out=t, in_=woT_d[cs * 128 : (cs + 1) * 128, :])
                wot.append(t)
            osb = {}
            for c in range(NCHUNKS):
                for cs in range(CH // 128):
                    osb[(c, cs)] = osb_pool.tile(
                        [128, NCHUNK], BF16, tag=f"osb{c}_{cs}", name=f"osb{c}_{cs}"
                    )

            # ---- filler emitters (projection chains on aux PSUM) ----
            def emit_v(ms):
                mp, ko = divmod(ms, 2)
                ps = aux_ps.tile([128, 512], F32, tag="aux", name="aux_ps_t")
                for k in range(KT):
                    nc.tensor.matmul(
                        ps[:, 0:CH],
                        xt[k][:, ms * 128 : (ms + 1) * 128],
                        wv[k],
                        start=(k == 0),
                        stop=(k == KT - 1),
                    )
                v4v = v4p[mp].rearrange("p (k h s) -> p k h s", k=2, h=HPG)
                if ko == 0:
                    nc.sync.dma_start(out=v4v[:, :, :, 64:65], in_=vones_d[:, :])
                nc.vector.tensor_add(
                    out=v4v[:, ko, :, 0:64],
                    in0=ps[:, 0:CH].rearrange("p (h c) -> p h c", c=64),
                    in1=bvb.rearrange("p (h c) -> p h c", c=64),
                )

            def emit_qk_chain(isq, cs, n0):
                dst, w, bias = (qt, wq, bq_t) if isq else (kt, wk, bk_t)
                ps = aux_ps.tile([128, 512], F32, tag="aux", name="aux_ps_t")
                for k in range(KT):
                    nc.tensor.matmul(
                        ps,
                        w[k][:, cs * 128 : (cs + 1) * 128],
                        xt[k][:, n0 : n0 + 512],
                        start=(k == 0),
                        stop=(k == KT - 1),
                    )
                nc.vector.tensor_scalar_add(
                    out=dst[cs][:, n0 : n0 + 512], in0=ps, scalar1=bias[cs]
                )

            def emit_f(c, msl):
                yp = aux_ps.tile([128, 512], F32, tag="aux", name="aux_yt_t")
                for cs in range(CH // 128):
                    nc.tensor.matmul(
                        yp,
                        wot[cs][:, msl * 128 : (msl + 1) * 128],
                        osb[(c, cs)],
                        start=(cs == 0),
                        stop=(cs == CH // 128 - 1),
                    )
                ysb = small.tile([128, 512], F16, tag="ysb", name="ysb_t", bufs=4)
                nc.vector.tensor_copy(out=ysb, in_=yp)
                nc.sync.dma_start(
                    out=yT_d[msl * 128 : (msl + 1) * 128, c * NCHUNK : (c + 1) * NCHUNK],
                    in_=ysb,
                )

            # ---- prelude: just what segment 0 needs to start ----
            emit_qk_chain(False, 0, 0)  # kt[0] cols 0:512 (ms 0..3)
            emit_qk_chain(True, 0, 0)   # qt[0] cols 0:512 (chunk 0)

            # ---- deadline-paced fillers, emitted AFTER scores+exp of their
            # slot (so score matmuls always lead in PE queue order) and
            # BEFORE the A*V matmul of their m-slice pair (so v4p[mp] writes
            # precede the DoubleRow matmul that reads them - Tile derives
            # dependencies from program order).
            def V(ms):
                return lambda: emit_v(ms)

            def QK(isq, cs, n0):
                return lambda: emit_qk_chain(isq, cs, n0)

            def F(c, msl):
                return lambda: emit_f(c, msl)

            # fillers keyed by m-slice PAIR: emitted after both exps of the
            # pair are queued (ScalarE has ~2.1us of buffered work to ride
            # out a chain) and before the pair's A*V (v4p[mp] deps).
            seg_fill = {
                0: {
                    0: [V(0), V(1)],
                    1: [V(2), V(3), QK(False, 0, 512)],
                    2: [V(4), V(5)],
                    3: [V(6), V(7), QK(False, 0, 1024)],
                    4: [V(8), V(9)],
                    5: [V(10), V(11), QK(False, 0, 1536)],
                    6: [V(12), V(13)],
                    7: [V(14), V(15), QK(True, 0, 512)],
                },
                1: {
                    0: [QK(True, 0, 1024)],
                    1: [QK(False, 1, 0)],
                    2: [QK(False, 1, 512)],
                    3: [QK(True, 0, 1536)],
                    4: [QK(False, 1, 1024)],
                    5: [QK(False, 1, 1536)],
                    6: [QK(True, 1, 0)],
                },
                2: {
                    0: [QK(True, 1, 512)],
                    2: [QK(True, 1, 1024)],
                    4: [QK(True, 1, 1536)],
                },
                3: {},
                4: {},
                5: {mp: [F(0, mp)] for mp in range(8)},
                6: {mp: [F(1, mp)] for mp in range(8)},
                7: {mp: [F(2, mp)] for mp in range(8)},
            }

            # ---- attention: pair-major segments ----
            seg = 0
            for cs in range(2):
                for c in range(NCHUNKS):
                    n0 = c * NCHUNK
                    fillers = seg_fill[seg]
                    ot = [
                        ot_pool.tile([65, NCHUNK], F32, tag=f"ot{hi}", name=f"ot{hi}_t")
                        for hi in range(2)
                    ]
                    for mp in range(MP):
                        et = et_pool.tile([128, 2048], FP8, tag="et", name="et_t")
                        for mi in range(2):
                            ms = 2 * mp + mi
                            st = st_pool.tile([128, 1024], F32, tag="st", name="st_t")
                            for hi in range(2):
                                r0 = hi * 64
                                nc.tensor.matmul(
                                    st[:, hi * 512 : (hi + 1) * 512],
                                    kt[cs][r0 : r0 + 64, ms * 128 : (ms + 1) * 128],
                                    qt[cs][r0 : r0 + 64, n0 : n0 + 512],
                                    start=True,
                                    stop=True,
                                )
                            nc.scalar.activation(
                                out=et[:, mi * 1024 : (mi + 1) * 1024],
                                in_=st,
                                func=mybir.ActivationFunctionType.Exp,
                                scale=float(1.0 / np.sqrt(DK)),
                            )
                        for f in fillers.get(mp, []):
                            f()
                        etv = et.rearrange("p (k n) -> p k n", k=2)
                        v4v = v4p[mp].rearrange("p (k s) -> p k s", k=2)
                        for hi in range(2):
                            h = 2 * cs + hi
                            nc.tensor.matmul(
                                ot[hi],
                                v4v[:, :, h * VPITCH : h * VPITCH + 65],
                                etv[:, :, hi * 512 : (hi + 1) * 512],
                                start=(mp == 0),
                                stop=(mp == MP - 1),
                                perf_mode=mybir.MatmulPerfMode.DoubleRow,
                            )
                    # normalize: drain ot, reciprocal of row-64 sums via a
                    # [128, 4] reshuffle, broadcast, multiply into osb rows.
                    for hi in range(2):
                        oraw = small.tile([65, NCHUNK], F32, tag="oraw", name="oraw_t")
                        nc.vector.tensor_copy(out=oraw, in_=ot[hi])
                        rcin = small.tile([128, NCHUNK // 128], F32, tag="rcin", name="rcin_t")
                        nc.sync.dma_start(out=rcin, in_=oraw[64:65, :])
                        rc = small.tile([128, NCHUNK // 128], F32, tag="rc", name="rc_t")
                        nc.vector.reciprocal(out=rc, in_=rcin)
                        rflat = small.tile([1, NCHUNK], F32, tag="rflat", name="rflat_t")
                        nc.sync.dma_start(out=rflat, in_=rc)
                        rb = small.tile([128, NCHUNK], F32, tag="rb", name="rb_t")
                        nc.gpsimd.partition_broadcast(rb, rflat)
                        nc.vector.tensor_mul(
                            out=osb[(c, cs)][hi * 64 : (hi + 1) * 64, :],
                            in0=oraw[0:64, :],
                            in1=rb[0:64, :],
                        )
                    seg += 1
            # epilogue: last chunk's output projection
            for msl in range(D // 128):
                emit_f(3, msl)
    nc.compile()
    return nc


_NC = None


def _get_nc():
    global _NC
    if _NC is None:
        _NC = _build_bass()
    return _NC


def build_in_maps(inputs):
    x = np.asarray(inputs["x"], dtype=np.float32)
    W_Q = np.asarray(inputs["W_Q"], dtype=np.float32)
    W_K = np.asarray(inputs["W_K"], dtype=np.float32)
    W_V = np.asarray(inputs["W_V"], dtype=np.float32)
    W_O = np.asarray(inputs["W_O"], dtype=np.float32)
    b_Q = np.asarray(inputs["b_Q"], dtype=np.float32)
    b_K = np.asarray(inputs["b_K"], dtype=np.float32)
    b_V = np.asarray(inputs["b_V"], dtype=np.float32)

    in_maps = []
    for core in range(NCORES):
        b, g = divmod(core, GROUPS)
        sl = slice(g * CH, (g + 1) * CH)
        in_maps.append(
            {
                "xT": np.ascontiguousarray(x[b].T.astype(ml_dtypes.bfloat16)),
                "wqT": np.ascontiguousarray(W_Q[sl, :].T.astype(ml_dtypes.bfloat16)),
                "wkT": np.ascontiguousarray(W_K[sl, :].T.astype(ml_dtypes.bfloat16)),
                "wvT": np.ascontiguousarray(W_V[sl, :].T.astype(ml_dtypes.bfloat16)),
                "woT": np.ascontiguousarray(W_O[:, sl].T.astype(ml_dtypes.bfloat16)),
                "bq": np.ascontiguousarray(b_Q[sl]),
                "bk": np.ascontiguousarray(b_K[sl]),
                "bv": np.ascontiguousarray(b_V[sl]),
                "vones": np.ones((128, 2 * HPG), dtype=ml_dtypes.float8_e4m3),
            }
        )
    return in_maps


def kernel(**inputs):
    in_maps = build_in_maps(inputs)
    nc = _get_nc()
    res = run_bass_kernel_spmd(nc, in_maps, core_ids=list(range(NCORES)))

    b_O = np.asarray(inputs["b_O"], dtype=np.float32)
    out = np.zeros((B, N, D), dtype=np.float32)
    for core in range(NCORES):
        b = core // GROUPS
        out[b] += res.results[core]["yT"].T
    out += b_O
    return out
